# revision 3
# baseline (speedup 1.0000x reference)
"""Embedding lookup on 8 Trainium2 NeuronCores — single-pass SWDGE dma_gather.

out[b, s, :] = W[:, input[b, s]]   (W: [d_model, vocab])

Data-parallel over tokens (2048/core); host converts the table to bf16
(rel-err gate 2e-2, bf16 rounds within 0.4%) and upcasts the result, so the
device moves half the bytes each way.

The gather uses the gpsimd dma_gather ucode (attnmlp library), which fetches
num_idxs arbitrary rows per instruction at ~994ns + 0.34ns/row — vs the
qPoolDynamic indirect-DMA path that caps at 128 rows (one per partition) per
~1.4us instruction. dma_gather indices are int16, and the ucode applies them
as SIGNED row offsets from the AP base, so the host packs idx-32768 and the
gather base points at table row 32768: the signed offset reaches the whole
50257-row vocab in one pass (verified on HW, including rows 0 and 50256).
The SWDGE descriptor ring holds 1024 descriptors, so the 2048 rows go as 2
ops of 1024. dma_gather lands row i at partition i%128 col i//128; the host
renumbers slots (i = (t%16)*128 + t//16) so SBUF holds token t = p*16+j at
[p, j], making out rows p*16+c*8..+8 16KB-contiguous per partition: each
store is one HWDGE op of 128x16KB descriptors. The library load runs before
the idx wait so the Q7 reload overlaps the idx DMA.
"""
import sys

sys.path.insert(0, "/opt/trn_rl_repo")

import contextlib

import ml_dtypes
import numpy as np

import concourse.bass as bass
from concourse import bacc, library_config, mybir
from concourse.bass_utils import run_bass_kernel_spmd

VOCAB = 50257
D_MODEL = 1024
BATCH = 4
SEQ = 4096
N_CORES = 8
P = 128
HALF = 32768

TOKENS = BATCH * SEQ              # 16384
T_CORE = TOKENS // N_CORES        # 2048 tokens per core
NT = T_CORE // P                  # 16 rows per partition
NCH = 2                           # gather/store chunks (ring: <=1024 desc/op)
TC = NT // NCH                    # 8 rows per chunk per partition

_compiled = None


def _build():
    # Strip the const-AP memsets + init-time all-engine barrier (this kernel
    # uses neither const_aps nor cross-engine state before its own sems), and
    # the monotonic-semaphore register machinery.
    orig_barrier = bass.Bass.all_engine_barrier
    orig_memset = bass.BassGpSimd.memset
    bass.Bass.all_engine_barrier = lambda self, **kw: None
    bass.BassGpSimd.memset = lambda self, *a, **kw: None
    try:
        nc = bacc.Bacc("TRN2", debug=False, num_devices=N_CORES,
                       monotonic_sem_count=0)
    finally:
        bass.Bass.all_engine_barrier = orig_barrier
        bass.BassGpSimd.memset = orig_memset
    table = nc.dram_tensor("table", [VOCAB, D_MODEL], mybir.dt.bfloat16,
                           kind="ExternalInput")
    idx16 = nc.dram_tensor("idx16", [P, T_CORE // 16], mybir.dt.int16,
                           kind="ExternalInput")
    out = nc.dram_tensor("out", [T_CORE, D_MODEL], mybir.dt.bfloat16,
                         kind="ExternalOutput")

    with contextlib.ExitStack() as st:
        idx_t = st.enter_context(
            nc.sbuf_tensor([P, T_CORE // 16], mybir.dt.int16))
        gbuf = st.enter_context(
            nc.sbuf_tensor([P, NT * D_MODEL], mybir.dt.bfloat16))
        i_sem = st.enter_context(nc.semaphore("i_sem"))
        g_sems = [st.enter_context(nc.semaphore(f"g{c}")) for c in range(NCH)]
        s_sem = st.enter_context(nc.semaphore("s_sem"))
        block = st.enter_context(nc.Block())

        # out rows p*NT + c*TC .. +TC are 16KB-contiguous per partition.
        out_v = out.ap().flatten().rearrange("(p u e) -> p u e", p=P, u=NCH)
        gbuf_v = gbuf[:, :].rearrange("p (j e) -> p j e", j=NT)
        tbase = table.ap()[HALF:, :]
        CW = T_CORE // 16 // NCH  # idx cols per chunk

        @block.sync
        def _(sync):
            sync.dma_start(idx_t[:, :], idx16.ap()).then_inc(i_sem, 16)
            for c in range(NCH):
                sync.wait_ge(g_sems[c], 16)
                sync.dma_start(out_v[:, c, :],
                               gbuf[:, c * TC * D_MODEL:(c + 1) * TC * D_MODEL]
                               ).then_inc(s_sem, 16)
            sync.wait_ge(s_sem, 16 * NCH)

        @block.gpsimd
        def _(gpsimd):
            gpsimd.load_library(library_config.attnmlp)
            gpsimd.wait_ge(i_sem, 16)
            for c in range(NCH):
                gpsimd.dma_gather(
                    out_ap=gbuf_v[:, c * TC:(c + 1) * TC, :],
                    in_ap=tbase,
                    idxs_ap=idx_t[:, c * CW:(c + 1) * CW],
                    num_idxs=T_CORE // NCH, num_idxs_reg=T_CORE // NCH,
                    elem_size=D_MODEL,
                ).then_inc(g_sems[c], 16)

    # Nothing here reads partition_id; prefill the caches so bass2jax's
    # cache_partition_id() emits no per-engine TENSOR_LOADs at program start.
    for eng in nc.engines.values():
        if eng._cached_partition_id is None:
            eng._cached_partition_id = 0
    nc._cached_partition_id_multi[tuple(mybir.ALL_ENGINES)] = 0
    nc.finalize()
    return nc


def _pack_idx(idx_core: np.ndarray) -> np.ndarray:
    """[T_CORE] int32 -> [128, T_CORE//16] int16, signed-rebased by -32768.

    Slot i holds token t(i) = (i%128)*16 + i//128 so the gather's
    (i%128, i//128) destination layout lands token t at [p=t//16, j=t%16].
    The wrap puts slot i at partition i%16 (replicated to p%16==i%16), col
    i//16 — the [channels, num_idxs//16] layout the Q7 ucode reads.
    """
    t = np.arange(T_CORE)
    i = (t % NT) * P + t // NT
    slots = np.empty(T_CORE, np.int64)
    slots[i] = idx_core
    s16 = (slots - HALF).astype(np.int16)
    m = np.arange(T_CORE // 16)
    ch = np.arange(P) % 16
    return np.ascontiguousarray(s16[(m[None, :] * 16) + ch[:, None]])


def prep_in_maps(input: np.ndarray, W: np.ndarray):
    table_np = np.ascontiguousarray(
        np.asarray(W, dtype=np.float32).T.astype(ml_dtypes.bfloat16))
    idx_flat = np.asarray(input, dtype=np.int32).reshape(TOKENS)
    return [
        {"table": table_np,
         "idx16": _pack_idx(idx_flat[k * T_CORE:(k + 1) * T_CORE])}
        for k in range(N_CORES)
    ]


def kernel(input: np.ndarray, W: np.ndarray) -> np.ndarray:
    global _compiled
    assert input.shape == (BATCH, SEQ) and W.shape == (D_MODEL, VOCAB)
    if _compiled is None:
        _compiled = _build()
    nc = _compiled

    in_maps = prep_in_maps(input, W)
    res = run_bass_kernel_spmd(nc, in_maps, core_ids=list(range(N_CORES)))
    out = np.concatenate(
        [np.asarray(res.results[k]["out"]) for k in range(N_CORES)], axis=0)
    return out.astype(np.float32).reshape(BATCH, SEQ, D_MODEL)


# revision 5
# speedup vs baseline: 1.3891x; 1.3891x over previous
"""Embedding lookup on 8 Trainium2 NeuronCores — bf16 bounce, eager stores.

out[b, s, :] = W[:, input[b, s]]   (W: [d_model, vocab])

Data-parallel over tokens (2048/core); host converts the table to bf16
(rel-err gate 2e-2, bf16 rounds within 0.4%) and upcasts the result, so the
device moves half the bytes each way. Per core: 16 SWDGE indirect gathers
of 128x2KB rows into SBUF. The Q7 descriptor-generation loop costs ~8ns per
descriptor (1.1us per 128-row op) and caps at one row per partition per op,
so the 16-op serial gen (~22.5us) is the critical path; the dma_gather ucode
alternative has the same per-row cost but adds a ~9.5us library load, so the
indirect path wins. Stores are per-tile (128x2KB descriptors) and issue the
moment each gather's completion sem fires, keeping writes flowing inside the
gen window's spare bus capacity and shrinking the post-gen drain tail to one
256KB store. Ramp trims: idx column 0 loads first so gather 0's generation
starts before the rest of the idx lands, and the unused partition-id caches
are prefilled so bass2jax's wrapper emits no per-engine TENSOR_LOADs at
program start.
"""
import sys

sys.path.insert(0, "/opt/trn_rl_repo")

import contextlib

import ml_dtypes
import numpy as np

import concourse.bass as bass
from concourse import mybir
from concourse.bass_utils import run_bass_kernel_spmd

VOCAB = 50257
D_MODEL = 1024
BATCH = 4
SEQ = 4096
N_CORES = 8
P = 128

TOKENS = BATCH * SEQ              # 16384
T_CORE = TOKENS // N_CORES        # 2048 tokens per core
NT = T_CORE // P                  # 16 gather ops of 128 rows

_compiled = None


def _build():
    # Strip the const-AP memsets + init-time all-engine barrier (this kernel
    # uses neither const_aps nor cross-engine state before its own sems), and
    # the monotonic-semaphore register machinery.
    orig_barrier = bass.Bass.all_engine_barrier
    orig_memset = bass.BassGpSimd.memset
    bass.Bass.all_engine_barrier = lambda self, **kw: None
    bass.BassGpSimd.memset = lambda self, *a, **kw: None
    try:
        nc = bass.Bass("TRN2", debug=False, num_devices=N_CORES,
                       monotonic_sem_count=0)
    finally:
        bass.Bass.all_engine_barrier = orig_barrier
        bass.BassGpSimd.memset = orig_memset
    table = nc.dram_tensor("table", [VOCAB, D_MODEL], mybir.dt.bfloat16,
                           kind="ExternalInput")
    idx = nc.dram_tensor("idx", [T_CORE], mybir.dt.int32, kind="ExternalInput")
    out = nc.dram_tensor("out", [T_CORE, D_MODEL], mybir.dt.bfloat16,
                         kind="ExternalOutput")

    with contextlib.ExitStack() as st:
        idx_tile = st.enter_context(nc.sbuf_tensor([P, NT], mybir.dt.int32))
        gbuf = st.enter_context(
            nc.sbuf_tensor([P, NT * D_MODEL], mybir.dt.bfloat16))
        idx_sem0 = st.enter_context(nc.semaphore("idx_sem0"))
        idx_sem1 = st.enter_context(nc.semaphore("idx_sem1"))
        g_sems = [st.enter_context(nc.semaphore(f"g{t}")) for t in range(NT)]
        s_sem = st.enter_context(nc.semaphore("s_sem"))
        block = st.enter_context(nc.Block())

        idx_v = idx.ap().rearrange("(p t) -> p t", p=P)
        # out row p*NT + t is one 2KB descriptor per partition per store.
        out_v = out.ap().flatten().rearrange("(p t e) -> p t e", p=P, t=NT)

        @block.sync
        def _(sync):
            sync.dma_start(idx_tile[:, 0:2], idx_v[:, 0:2]).then_inc(idx_sem0, 16)
            sync.dma_start(idx_tile[:, 2:NT], idx_v[:, 2:NT]).then_inc(idx_sem1, 16)
            for t in range(0, NT, 2):
                sync.wait_ge(g_sems[t], 16)
                sync.dma_start(out_v[:, t, :],
                               gbuf[:, t * D_MODEL:(t + 1) * D_MODEL]
                               ).then_inc(s_sem, 16)
            sync.wait_ge(s_sem, 16 * NT)

        @block.scalar
        def _(scalar):
            for t in range(1, NT, 2):
                scalar.wait_ge(g_sems[t], 16)
                scalar.dma_start(out_v[:, t, :],
                                 gbuf[:, t * D_MODEL:(t + 1) * D_MODEL]
                                 ).then_inc(s_sem, 16)

        @block.gpsimd
        def _(gpsimd):
            gpsimd.wait_ge(idx_sem0, 16)
            for t in range(NT):
                if t == 2:
                    gpsimd.wait_ge(idx_sem1, 16)
                gpsimd.indirect_dma_start(
                    out=gbuf[:, t * D_MODEL:(t + 1) * D_MODEL],
                    out_offset=None,
                    in_=table.ap(),
                    in_offset=bass.IndirectOffsetOnAxis(
                        ap=idx_tile[:, t:t + 1], axis=0),
                ).then_inc(g_sems[t], 16)

    # Nothing here reads partition_id; prefill the caches so bass2jax's
    # cache_partition_id() emits no per-engine TENSOR_LOADs at program start.
    for eng in nc.engines.values():
        if eng._cached_partition_id is None:
            eng._cached_partition_id = 0
    nc._cached_partition_id_multi[tuple(mybir.ALL_ENGINES)] = 0
    return nc


def prep_in_maps(input: np.ndarray, W: np.ndarray):
    table_np = np.ascontiguousarray(
        np.asarray(W, dtype=np.float32).T.astype(ml_dtypes.bfloat16))
    idx_flat = np.ascontiguousarray(
        np.asarray(input, dtype=np.int32).reshape(TOKENS))
    return [
        {"table": table_np, "idx": idx_flat[k * T_CORE:(k + 1) * T_CORE]}
        for k in range(N_CORES)
    ]


def kernel(input: np.ndarray, W: np.ndarray) -> np.ndarray:
    global _compiled
    assert input.shape == (BATCH, SEQ) and W.shape == (D_MODEL, VOCAB)
    if _compiled is None:
        _compiled = _build()
    nc = _compiled

    in_maps = prep_in_maps(input, W)
    res = run_bass_kernel_spmd(nc, in_maps, core_ids=list(range(N_CORES)))
    out = np.concatenate(
        [np.asarray(res.results[k]["out"]) for k in range(N_CORES)], axis=0)
    return out.astype(np.float32).reshape(BATCH, SEQ, D_MODEL)
